# revision 64
# baseline (speedup 1.0000x reference)
"""CenterLoss kernel for Trainium2 (8 NeuronCores, SPMD data-parallel over B).

Algorithm
---------
reference computes:
    counts[c] = #{i: y_i = c};  sums[c] = sum_{i: y_i = c} f_i
    means = sums / max(counts, 1);  present = counts > 0
    n_c = present ? 0.5*centers_c + 0.5*means_c : centers_c
    loss = 0.5 * mean_i ||f_i - n_{y_i}||^2

Expanding the loss (every class that appears in the batch is present):
    B * 2 * loss = S1 - 0.5*A - 0.75*X + 0.25*W
where
    S1 = sum_i ||f_i||^2
    A  = sum_c sums_c . centers_c
    X  = sum_{c present} ||sums_c||^2 / counts_c
    W  = sum_c counts_c * ||centers_c||^2

So the only heavy device work is the segment sums/counts over feats
(B=131072, D=256, C=1000) and S1.  Each core takes B/8 rows and computes:
  - partial segment sums+counts via one-hot matmuls on the PE
    (one-hot built on DVE from an iota table, feats converted fp32->fp16 on
    ACT; counts ride along as a 257th all-ones column of the rhs)
  - partial S1 via ACT Square activation with free-dim accumulation
The host sums the 8 partial [1024,257] tensors + 8 partial S1 vectors and
evaluates the tiny [C,D] closed form above (the gather/unshard step).
"""

import sys

sys.path.insert(0, "/opt/trn_rl_repo")

import numpy as np

# problem shape (hardcoded per the harness contract)
B, D, C = 131072, 256, 1000
N_CORES = 8
BS = B // N_CORES  # 16384 rows per core
P = 128
G = 4  # row-tiles per DMA group
TILES = BS // P  # 128
GROUPS = TILES // G  # 32
CPAD = 1024  # padded class count
CCHUNKS = CPAD // P  # 8
NFREE = D + 1  # 256 feat cols + 1 ones col for counts
FSTRIDE = 264  # fp16 sub-tile stride (4B aligned, 16B padded)
TAILG = 4  # trailing groups processed chunk-outer (store/compute overlap)

_CACHE: dict = {}


def _build_program():
    import concourse.bacc as bacc
    import concourse.bass as bass
    from concourse import mybir
    from concourse.tile import TileContext

    nc = bacc.Bacc("TRN2", target_bir_lowering=False)

    feats = nc.dram_tensor("feats", [BS, D], mybir.dt.float32, kind="ExternalInput")
    labels_in = nc.dram_tensor(
        "labels", [P, TILES], mybir.dt.float16, kind="ExternalInput"
    )
    # [128 x (8*257 sums+counts | 1 s1)]; stored per chunk so early stores
    # overlap the tail matmuls
    out_sums = nc.dram_tensor(
        "out_sums", [P, CCHUNKS * NFREE + 1], mybir.dt.float32, kind="ExternalOutput"
    )

    feats_ap = feats[:]

    with TileContext(nc) as tc:
        with (
            tc.tile_pool(name="const", bufs=1) as const,
            tc.tile_pool(name="fin", bufs=4) as fin,
            tc.tile_pool(name="sq", bufs=2) as sqp,
            tc.tile_pool(name="f16p", bufs=TAILG + 2) as f16p,
            tc.tile_pool(name="ohp", bufs=4 * TAILG + 6) as ohp,
            tc.tile_pool(name="accp", bufs=1) as accp,
            tc.tile_pool(name="psp", bufs=1, space="PSUM") as psp,
        ):
            # labels DMA (fp16, converted to fp32 on DVE: tensor_scalar
            # is_equal needs an fp32 scalar operand); iota built on the
            # otherwise-idle GPSIMD engine, converted int32 -> fp16 on DVE
            labels16_t = const.tile([P, TILES], mybir.dt.float16, tag="labels16_t")
            nc.sync.dma_start(out=labels16_t[:], in_=labels_in[:])
            labels_t = const.tile([P, TILES], mybir.dt.float32, tag="labels_t")
            nc.vector.tensor_copy(out=labels_t[:], in_=labels16_t[:])
            iota_i = const.tile([P, CPAD], mybir.dt.int32, tag="iota_i")
            nc.gpsimd.iota(iota_i[:], pattern=[[1, CPAD]], channel_multiplier=0)
            iota_f = const.tile([P, CPAD], mybir.dt.float16, tag="iota_f")
            nc.vector.tensor_copy(out=iota_f[:], in_=iota_i[:])
            iota_t = iota_f[:]

            tail_ohs, tail_f16gs = [], []
            # persistent accumulators
            # one column per (group, extra-half): 32 + 3 split extras
            s1cols = accp.tile([P, GROUPS + 3], mybir.dt.float32, tag="s1cols")
            s1_extra_col = [GROUPS]  # next free extra column
            psums = [
                psp.tile(
                    [P, NFREE], mybir.dt.float32, tag=f"psum{k}", name=f"psum{k}"
                )
                for k in range(CCHUNKS)
            ]
            # HAM warm-up: the PE runs at the cold 1.2 GHz clock until ~3.4us
            # of sustained activity. The head leaves PE idle until ~4.6us, so
            # the first ~19 real matmuls would run at half clock. Issue dummy
            # matmuls (zeroed operands, results discarded by the real
            # start=True PSUM clear) from ~0.5us so the real stream is warm.
            warm = const.tile([P, NFREE], mybir.dt.float16, tag="warm")
            nc.vector.memset(warm[:1, :1], 0.0)  # touch so Tile allocates it
            for w in range(12):
                nc.tensor.matmul(
                    out=psums[0][:],
                    lhsT=warm[:, 0:P],
                    rhs=warm[:],
                    start=True,
                    stop=True,
                )

            for t in range(GROUPS):
                # load a [P, G, D] group of feats rows (rows t*512 .. t*512+511).
                # Groups 0/1 are split into smaller loads/conversions so the
                # first matmul starts as soon as the first 128 rows land.
                f16g = f16p.tile([P, G, FSTRIDE], mybir.dt.float16, tag="f16g")
                if t == 0:
                    halves = ((0, 1), (1, 1), (2, 2))
                elif t == 1:
                    halves = ((0, 2), (2, 2))
                else:
                    halves = ((0, G),)
                for h, (off, gh) in enumerate(halves):
                    fg = fin.tile(
                        [P, gh, D], mybir.dt.float32, tag="fg", name="fg"
                    )
                    # very first load rides the ACT HWDGE ring so its
                    # descriptor-gen overlaps the labels DMA's on the SP ring
                    dma_eng = nc.scalar if t == 0 else nc.sync
                    dma_eng.dma_start(
                        out=fg[:],
                        in_=bass.AP(
                            tensor=feats_ap.tensor,
                            offset=(t * G + off) * P * D,
                            ap=[[D, P], [P * D, gh], [1, D]],
                        ),
                    )
                    # fp32 -> fp16 conversion (ACT)
                    nc.scalar.copy(
                        out=f16g[:, off : off + gh, 0:D], in_=fg[:]
                    )
                    # S1 partial: sum over free dim of feats^2 (ACT square+accum)
                    sqt = sqp.tile([P, gh, D], mybir.dt.float32, tag="sqt", name="sqt")
                    if h == 0:
                        col = t
                    else:
                        col = s1_extra_col[0]
                        s1_extra_col[0] += 1
                    nc.scalar.activation(
                        out=sqt[:],
                        in_=fg[:],
                        func=mybir.ActivationFunctionType.Square,
                        accum_out=s1cols[:, col : col + 1],
                    )
                # ones column for counts (DVE)
                nc.vector.memset(f16g[:, :, D : D + 1], 1.0)

                ohs = []
                for s in range(G):
                    j = t * G + s
                    oh = ohp.tile([P, CPAD], mybir.dt.float16, tag="oh")
                    nc.vector.tensor_scalar(
                        oh[:],
                        iota_t,
                        labels_t[:, j : j + 1],
                        None,
                        mybir.AluOpType.is_equal,
                    )
                    ohs.append(oh)
                if t < GROUPS - TAILG:
                    for s in range(G):
                        rhs = f16g[:, s, 0:NFREE]
                        for k in range(CCHUNKS):
                            nc.tensor.matmul(
                                out=psums[k][:],
                                lhsT=ohs[s][:, k * P : (k + 1) * P],
                                rhs=rhs,
                                start=(t == 0 and s == 0),
                                stop=False,
                            )
                else:
                    tail_ohs.append(ohs)
                    tail_f16gs.append(f16g)
            # last TAILG groups: chunk-outer order so chunk k's accumulation
            # closes early and its evacuation/store overlaps the remaining
            # chunks' matmuls
            for k in range(CCHUNKS):
                for g, (ohs_g, f16g_g) in enumerate(zip(tail_ohs, tail_f16gs)):
                    for s in range(G):
                        nc.tensor.matmul(
                            out=psums[k][:],
                            lhsT=ohs_g[s][:, k * P : (k + 1) * P],
                            rhs=f16g_g[:, s, 0:NFREE],
                            start=False,
                            stop=(g == TAILG - 1 and s == G - 1),
                        )

            # write back partials (PSUM -> SBUF -> DRAM; DMA can't read PSUM)
            ev = accp.tile([P, CCHUNKS * NFREE + 1], mybir.dt.float32, tag="ev")
            nc.vector.tensor_reduce(
                out=ev[:, CCHUNKS * NFREE : CCHUNKS * NFREE + 1],
                in_=s1cols[:],
                axis=mybir.AxisListType.X,
                op=mybir.AluOpType.add,
            )
            for k in range(CCHUNKS):
                dst = ev[:, k * NFREE : (k + 1) * NFREE]
                if k % 2 == 0:
                    nc.vector.tensor_copy(out=dst, in_=psums[k][:])
                else:
                    nc.scalar.copy(out=dst, in_=psums[k][:])
            # per-chunk stores: chunks close ~1.7 us apart (chunk-outer tail),
            # so early stores hide under compute and the last piece is small
            for k in range(CCHUNKS):
                lo = k * NFREE
                hi = (k + 1) * NFREE + (1 if k == CCHUNKS - 1 else 0)
                nc.sync.dma_start(out=out_sums[:, lo:hi], in_=ev[:, lo:hi])

    nc.compile()
    return nc


def _get_program():
    if "nc" not in _CACHE:
        _CACHE["nc"] = _build_program()
    return _CACHE["nc"]


def _run_device(feats_np: np.ndarray, labels_np: np.ndarray, trace: bool = False):
    """Shard over cores, run the SPMD bass kernel, return per-core results."""
    from concourse.bass_utils import run_bass_kernel_spmd

    nc = _get_program()
    in_maps = []
    for c in range(N_CORES):
        fshard = np.ascontiguousarray(feats_np[c * BS : (c + 1) * BS])
        lshard = labels_np[c * BS : (c + 1) * BS]
        # [P, TILES]; fp16 is exact for labels < 2048
        ltile = np.ascontiguousarray(lshard.reshape(TILES, P).T.astype(np.float16))
        in_maps.append({"feats": fshard, "labels": ltile})
    kw = {}
    if trace:
        kw = {"trace": True}
    try:
        return run_bass_kernel_spmd(nc, in_maps, core_ids=list(range(N_CORES)), **kw)
    except Exception:
        # transient axon/terminal faults have been observed; retry once
        import time

        time.sleep(2.0)
        return run_bass_kernel_spmd(nc, in_maps, core_ids=list(range(N_CORES)), **kw)


def kernel(feats, centers, labels, _trace: bool = False, _return_res: bool = False):
    feats = np.asarray(feats, dtype=np.float32)
    centers = np.asarray(centers, dtype=np.float32)
    labels_i = np.asarray(labels).astype(np.int64)

    res = _run_device(feats, labels_i, trace=_trace)

    # host combine (the gather/unshard step): tiny [C, D] math
    sums_all = np.zeros((CPAD, NFREE), dtype=np.float64)
    S1 = 0.0
    for c in range(N_CORES):
        raw = res.results[c]["out_sums"]
        part = (
            raw[:, : CCHUNKS * NFREE]
            .reshape(P, CCHUNKS, NFREE)
            .transpose(1, 0, 2)
            .reshape(CPAD, NFREE)
        )
        sums_all += part.astype(np.float64)
        S1 += float(raw[:, CCHUNKS * NFREE].sum())
    sums = sums_all[:C, :D]
    counts = sums_all[:C, D]

    c64 = centers.astype(np.float64)
    A = float((sums * c64).sum())
    present = counts > 0
    X = float((np.square(sums).sum(axis=1)[present] / counts[present]).sum())
    W = float((counts * np.square(c64).sum(axis=1)).sum())
    loss = 0.5 / B * (S1 - 0.5 * A - 0.75 * X + 0.25 * W)
    out = np.float32(loss)
    if _return_res:
        return out, res
    return out
